# revision 2
# baseline (speedup 1.0000x reference)
"""
Trainium2 Bass kernel for nn_Encoder (embedding lookup + LSTM -> final (h, c)).

Data-parallel over batch: 8 cores x 4 batch rows each. Per core, per step:
  - ONE psum bank tile pz [128, 64]; col = q*16 + slot(G)*4 + b, where quad
    q = h-slice (128 rows of h), slot order (g, f, i, o).
  - xz_t injected into psum by an fp16 identity matmul (start=True), so the
    gate pre-activations accumulate fully inside PSUM (no DVE add).
  - 64 (LDWEIGHTS + MATMUL[N=4]) pairs in k-major sweeps; the last sweep
    orders quads 0..3 so their elementwise chains start staggered.
  - All gate nonlinearities via tanh only: sigmoid(z) = 0.5 + 0.5*tanh(z/2),
    the 1/2 folded into the U/W/b columns of i,f,o at weight-staging time.
  - State kept scaled: s = 2c and h16 = 2h (compensated by an extra 1/2 on
    the stationary U), which lets scalar_tensor_tensor fuse each chain into:
      ta:   a = tanh(scale * z)            (ACT, 16 cols, reads psum)
      op1:  w8 = [(1+a_f)*s | (1+a_i)*a_g] (DVE, fused via [s|a_g] layout)
      op2:  s' = 0.5*w8.lo + w8.hi         (DVE, written to next parity slot)
      tc:   tanh(s'/2)                     (ACT)
      mulh: h2 = (1+a_o)*tc                (DVE -> h16, other parity half)
  - Chain ops batched 1+3: quad 0 solo (it gates the critical cycle),
    quads 1-3 in wide strided ops. Work/state tiles are parity double-
    buffered (wqt, h16) to kill cross-step WAR serialization.
  - Prep (token gather + PE transpose + x@W projection into packed fp16 xz)
    runs as a generator polled every 4 steps so its PE work lands in the
    inter-step stall slots.

Host side: shard batch across 8 cores, run SPMD, unpack (h, c) (divide by 2).
"""

import numpy as np

import concourse.bass as bass
import concourse.mybir as mybir
import concourse.tile as tile
from concourse import bacc
from concourse.bass import IndirectOffsetOnAxis
from concourse.bass_utils import run_bass_kernel_spmd
from concourse.masks import make_identity

B, T, V, E, H = 32, 512, 20000, 300, 512
G4 = 4 * H
NCORES = 8
BL = B // NCORES
P = 128
KM = G4 // P          # 16 gate-block tiles
KH = H // P           # 4 k tiles over H
KE_SIZES = [128, 128, 44]
# gate G (Keras order i,f,g,o) -> slot within a quad's 16 cols (g,f,i,o)
SLOT = [2, 1, 0, 3]
WQ = 32               # work cols per quad

f32 = mybir.dt.float32
f16 = mybir.dt.float16
i32 = mybir.dt.int32
AF = mybir.ActivationFunctionType
ALU = mybir.AluOpType

STAT_DT = {"f16": f16, "f8e3": mybir.dt.float8e3, "f8e4": mybir.dt.float8e4}
STAT_SU = {"f16": 1.0, "f8e3": 64.0, "f8e4": 32.0}


def _sched(spec):
    out = []
    for tok in spec.split():
        kind = tok[:-1]
        out.append((kind, int(tok[-1])))
    return out


# Chain emission schedules: per-engine FIFO order is emission order, so the
# order controls cross-quad overlap. ta/tc -> ACT, dve (3-op block) -> DVE,
# mulh -> DVE (or Pool if CHAIN_MULH_POOL).
CHAIN_SCHEDS = {
    # naive per-quad (serializes chains)
    "A": _sched("ta0 dve0 tc0 mulh0 ta1 dve1 tc1 mulh1 "
                "ta2 dve2 tc2 mulh2 ta3 dve3 tc3 mulh3"),
    # balanced interleave
    "B": _sched("ta0 ta1 dve0 tc0 ta2 dve1 mulh0 tc1 ta3 dve2 mulh1 "
                "tc2 dve3 mulh2 tc3 mulh3"),
    # tas early, tcs late
    "C": _sched("ta0 ta1 dve0 ta2 dve1 tc0 ta3 dve2 mulh0 tc1 dve3 "
                "mulh1 tc2 mulh2 tc3 mulh3"),
    # fully stage-major
    "D": _sched("ta0 ta1 ta2 ta3 dve0 dve1 dve2 dve3 tc0 tc1 tc2 tc3 "
                "mulh0 mulh1 mulh2 mulh3"),
}
CHAIN_SCHED = CHAIN_SCHEDS["B"]
CHAIN_MULH_POOL = False
CHAIN_NOSYNC = True
CHAIN_SPLIT13 = True
PREP_POOL = False
NO_CHAIN = False
TAIL2 = False


def build_program_v3(nc, T_steps=T, Tc=128, reps=1, stat="f16", dbg_step=None):
    assert T_steps % Tc == 0
    nch = T_steps // Tc
    NJ = Tc * BL // P  # 128-row gathers per chunk
    sdt = STAT_DT[stat]
    su = STAT_SU[stat]

    emb_t = nc.declare_dram_parameter("emb", [V, E], f32, isOutput=False)
    W_t = nc.declare_dram_parameter("W", [E, G4], f32, isOutput=False)
    U_t = nc.declare_dram_parameter("U", [H, G4], f32, isOutput=False)
    b_t = nc.declare_dram_parameter("bvec", [G4], f32, isOutput=False)
    tok_t = nc.declare_dram_parameter("tok", [P, T_steps * BL // P], i32, isOutput=False)
    ho_t = nc.declare_dram_parameter("ho", [P, BL * KH], f16, isOutput=True)
    co_t = nc.declare_dram_parameter("co", [P, BL * KH], f32, isOutput=True)
    if dbg_step is not None:
        dbg_w = nc.declare_dram_parameter("dbg_w", [P, 4 * WQ], f32, isOutput=True)
        dbg_h = nc.declare_dram_parameter("dbg_h", [P, BL * KH], f16, isOutput=True)
        dbg_z = nc.declare_dram_parameter("dbg_z", [P, 64], f32, isOutput=True)

    with tile.TileContext(nc) as tc:
        with (
            tc.tile_pool(name="const", bufs=1) as cpool,
            tc.tile_pool(name="ustage", bufs=2) as upool,
            tc.tile_pool(name="xrows", bufs=4) as xpool,
            tc.tile_pool(name="xtp", bufs=2) as xtpool,
            tc.tile_pool(name="ptr", bufs=2, space="PSUM") as ptr_pool,
            tc.tile_pool(name="pxz", bufs=2, space="PSUM") as pxz_pool,
            tc.tile_pool(name="pz", bufs=4, space="PSUM") as pz_pool,
        ):
            US = cpool.tile([P, KH * G4], sdt, tag="US")
            W_sb = cpool.tile([P, 3 * G4], f16, tag="Wsb")
            b_sb = cpool.tile([P, KM], f32, tag="bsb")
            tok_sb = cpool.tile([P, T_steps * BL // P], i32, tag="tok")
            ident32 = cpool.tile([P, P], f32, tag="id32")
            identS = cpool.tile([P, P], sdt, tag="idS")
            # h16 double-buffered by step parity: cols pp*16 + k*4 + b
            h16 = cpool.tile([P, 2 * BL * KH], f16, tag="h16")
            # Single work tile, two parity halves of 4*WQ cols each. Per-quad
            # block (parity pp, quad q) at pp*4*WQ + q*WQ:
            #   0-3 s(=2c), 4-7 a_g, 8-11 a_f, 12-15 a_i, 16-19 a_o,
            #   20-27 w8=[t1|t2], 28-31 tc.
            # s' is written to BOTH parity s-slots so [s | a_g] is contiguous.
            wqt = cpool.tile([P, 2 * 4 * WQ], f32, tag="wqt")
            co_sb = cpool.tile([P, BL * KH], f32, tag="co")
            xz_sb = [
                cpool.tile([P, Tc * 64], f16, tag=f"xz{par}", name=f"xz{par}")
                for par in range(2)
            ]

            make_identity(nc, ident32[:])
            make_identity(nc, identS[:])

            # U (f32 DRAM) -> US (scaled cast). Column blocks m: 0-7 i,f
            # (sigma=1/2), 8-11 g (sigma=1), 12-15 o (1/2). Extra 1/2 because
            # h16 stores 2h (see chain).
            SPLITS = [(0, 8 * P, 0.5), (8 * P, 12 * P, 1.0), (12 * P, 16 * P, 0.5)]
            for k in range(KH):
                ust = upool.tile([P, G4], f32, tag="ustage")
                nc.sync.dma_start(ust[:], U_t.ap()[k * P:(k + 1) * P, :])
                for c0, c1, sg in SPLITS:
                    nc.vector.tensor_scalar_mul(
                        US[:, k * G4 + c0:k * G4 + c1], ust[:, c0:c1],
                        sg * su * 0.5
                    )

            # W: 3 E-subtiles, scaled cast to fp16
            ofs = 0
            for kk, kw in enumerate(KE_SIZES):
                wst = upool.tile([P, G4], f32, tag="ustage", name=f"wst{kk}")
                nc.sync.dma_start(wst[:kw, :], W_t.ap()[ofs:ofs + kw, :])
                for c0, c1, sg in SPLITS:
                    nc.vector.tensor_scalar_mul(
                        W_sb[:kw, kk * G4 + c0:kk * G4 + c1], wst[:kw, c0:c1], sg * su
                    )
                ofs += kw

            # bias: b_sb[p, m] = b[m*128 + p] * scale(m)
            nc.sync.dma_start(b_sb[:], b_t.ap().rearrange("(m p) -> p m", p=P))
            for c0, c1, sg in [(0, 8, 0.5), (8, 12, 1.0), (12, 16, 0.5)]:
                nc.vector.tensor_scalar_mul(
                    b_sb[:, c0:c1], b_sb[:, c0:c1], sg * su
                )
            nc.sync.dma_start(tok_sb[:], tok_t.ap())

            nc.gpsimd.memset(h16[:], 0.0)
            nc.gpsimd.memset(wqt[:], 0.0)

            def PREP_ENG():
                return nc.gpsimd if PREP_POOL else nc.vector

            def prep_gen(c):
                """Generator: gather + transpose + xz projection for chunk c."""
                xz_dst = xz_sb[c % 2]
                xT = xtpool.tile([P, 3 * Tc * BL], f16, tag="xT")
                N = Tc * BL
                for j in range(NJ):
                    xr = xpool.tile([P, E], f32, tag="xrows")
                    nc.gpsimd.indirect_dma_start(
                        out=xr[:],
                        out_offset=None,
                        in_=emb_t.ap(),
                        in_offset=IndirectOffsetOnAxis(
                            ap=tok_sb[:, c * NJ + j:c * NJ + j + 1], axis=0
                        ),
                    )
                    yield
                    for kk, kw in enumerate(KE_SIZES):
                        pt = ptr_pool.tile([P, P], f32, tag="ptr")
                        nc.tensor.transpose(
                            out=pt[:kw, :], in_=xr[:, kk * P:kk * P + kw],
                            identity=ident32[:],
                        )
                        PREP_ENG().tensor_copy(
                            xT[:kw, kk * N + j * P:kk * N + (j + 1) * P],
                            pt[:kw, :],
                        )
                        yield
                for m in range(KM):
                    pxz = pxz_pool.tile([P, N], f32, tag="pxz")
                    for kk, kw in enumerate(KE_SIZES):
                        nc.tensor.matmul(
                            pxz[:],
                            W_sb[:kw, kk * G4 + m * P:kk * G4 + (m + 1) * P],
                            xT[:kw, kk * N:(kk + 1) * N],
                            start=(kk == 0),
                            stop=(kk == 2),
                        )
                    slot = (m % 4) * 16 + SLOT[m // 4] * 4
                    dst = xz_dst[:].rearrange("p (t g) -> p t g", g=64)[
                        :, :, slot:slot + 4
                    ]
                    src = pxz[:].rearrange("p (t b) -> p t b", b=BL)
                    PREP_ENG().tensor_scalar_add(dst, src, b_sb[:, m:m + 1])
                    yield

            def chain_ops(q, pz, pp):
                """Closures for quad q's chain stages (parity pp in {0,1})."""
                w0 = pp * 4 * WQ + q * WQ

                def ta():
                    # a = tanh(sigma*z); psum holds su*sigma*z
                    return nc.scalar.activation(
                        wqt[:, w0 + 4:w0 + 20], pz[:, q * 16:q * 16 + 16],
                        AF.Tanh, scale=1.0 / su,
                    )

                def dve():
                    # w8 = [(1+a_f)*s | (1+a_i)*a_g] = [4*fhat*c | 2*ihat*g]
                    nc.vector.scalar_tensor_tensor(
                        wqt[:, w0 + 20:w0 + 28], wqt[:, w0 + 8:w0 + 16],
                        1.0, wqt[:, w0 + 0:w0 + 8], ALU.add, ALU.mult,
                    )
                    # s' = 0.5*t1 + t2 = 2c', written to the OTHER parity's
                    # s-slot (that's where step t+1 reads [s | a_g] from)
                    w1 = (1 - pp) * 4 * WQ + q * WQ
                    nc.vector.scalar_tensor_tensor(
                        wqt[:, w1 + 0:w1 + 4], wqt[:, w0 + 20:w0 + 24],
                        0.5, wqt[:, w0 + 24:w0 + 28], ALU.mult, ALU.add,
                    )

                def tc():
                    # tc = tanh(s'/2) = tanh(c'); s' lives at other parity
                    w1 = (1 - pp) * 4 * WQ + q * WQ
                    return nc.scalar.activation(
                        wqt[:, w0 + 28:w0 + 32], wqt[:, w1 + 0:w1 + 4],
                        AF.Tanh, scale=0.5,
                    )

                def mulh():
                    # h2 = (1+a_o)*tc = 2h (fp16), into the OTHER parity half
                    hb = (1 - pp) * BL * KH
                    eng = nc.gpsimd if CHAIN_MULH_POOL else nc.vector
                    eng.scalar_tensor_tensor(
                        h16[:, hb + q * BL:hb + (q + 1) * BL],
                        wqt[:, w0 + 16:w0 + 20],
                        1.0, wqt[:, w0 + 28:w0 + 32], ALU.add, ALU.mult,
                    )

                return {"ta": ta, "dve": dve, "tc": tc, "mulh": mulh}

            def emit_chain13(pz, pp):
                """Chains with quad 0 solo (early, on the critical cycle) and
                quads 1-3 batched into wide strided ops (slack absorbs it)."""
                base = pp * 4 * WQ
                obase = (1 - pp) * 4 * WQ
                hb = (1 - pp) * BL * KH
                w3 = lambda off, w=4: wqt[:].rearrange(
                    "p (blk x) -> p blk x", x=WQ
                )[:, pp * 4 + 1:pp * 4 + 4, off:off + w]
                w3o = lambda off, w=4: wqt[:].rearrange(
                    "p (blk x) -> p blk x", x=WQ
                )[:, (1 - pp) * 4 + 1:(1 - pp) * 4 + 4, off:off + w]
                # ta0 (16c) then ta123 (48c)
                nc.scalar.activation(
                    wqt[:, base + 4:base + 20], pz[:, 0:16], AF.Tanh,
                    scale=1.0 / su,
                )
                nc.scalar.activation(
                    w3(4, 16), pz[:].rearrange("p (qq cc) -> p qq cc", cc=16)[
                        :, 1:4, :
                    ], AF.Tanh, scale=1.0 / su,
                )
                # DVE pairs interleaved: q123's op1 fills the same-engine
                # sem bubble between q0's op1 and op2
                nc.vector.scalar_tensor_tensor(
                    wqt[:, base + 20:base + 28], wqt[:, base + 8:base + 16],
                    1.0, wqt[:, base + 0:base + 8], ALU.add, ALU.mult,
                )
                nc.vector.scalar_tensor_tensor(
                    w3(20, 8), w3(8, 8), 1.0, w3(0, 8), ALU.add, ALU.mult,
                )
                nc.vector.scalar_tensor_tensor(
                    wqt[:, obase + 0:obase + 4], wqt[:, base + 20:base + 24],
                    0.5, wqt[:, base + 24:base + 28], ALU.mult, ALU.add,
                )
                nc.vector.scalar_tensor_tensor(
                    w3o(0, 4), w3(20, 4), 0.5, w3(24, 4), ALU.mult, ALU.add,
                )
                # tc0 + mulh0
                nc.scalar.activation(
                    wqt[:, base + 28:base + 32], wqt[:, obase + 0:obase + 4],
                    AF.Tanh, scale=0.5,
                )
                nc.vector.scalar_tensor_tensor(
                    h16[:, hb + 0:hb + BL], wqt[:, base + 16:base + 20],
                    1.0, wqt[:, base + 28:base + 32], ALU.add, ALU.mult,
                )
                # tc123 + mulh123
                nc.scalar.activation(
                    w3(28, 4), w3o(0, 4), AF.Tanh, scale=0.5,
                )
                nc.vector.scalar_tensor_tensor(
                    h16[:, hb + BL:hb + 4 * BL], w3(16, 4),
                    1.0, w3(28, 4), ALU.add, ALU.mult,
                )

            def emit_step(c, t, gen):
                pz = pz_pool.tile([P, 512], f32, tag="pz", name=f"pz_{c}_{t}")
                pp = t % 2
                # inject xz_t into psum (also the accumulation-group start)
                nc.tensor.matmul(
                    pz[:, 0:64], identS[:],
                    xz_sb[c % 2][:, t * 64:(t + 1) * 64],
                    start=True, stop=False, skip_group_check=True,
                )
                if gen is not None and t % 4 == 2:
                    next(gen, None)
                hb = pp * BL * KH

                def mm(k, q, G):
                    m = 4 * G + q
                    nc.tensor.matmul(
                        pz[:, q * 16 + SLOT[G] * 4:q * 16 + SLOT[G] * 4 + 4],
                        US[:, k * G4 + m * P:k * G4 + (m + 1) * P],
                        h16[:, hb + k * BL:hb + (k + 1) * BL],
                        start=False, stop=(k == KH - 1),
                        skip_group_check=True,
                    )

                if TAIL2 and not NO_CHAIN:
                    # k0, k1 full sweeps; then per-quad (k2, k3) tails. Chains
                    # start ~2 sweeps earlier and overlap later quads' MMs.
                    # Emission interleaved in readiness order; nosync hints pin
                    # the ACT order to [ta0 ta1 tc0 ta2 tc1 ta3 tc2 tc3].
                    from concourse.bass import InstructionNameOrderedSet

                    for k in (0, 1):
                        for q in range(4):
                            for G in range(4):
                                mm(k, q, G)
                    ops = [chain_ops(q, pz, pp) for q in range(4)]

                    def tail(q):
                        for k in (2, 3):
                            for G in range(4):
                                mm(k, q, G)

                    tas = {}

                    def tc_after(q, qa):
                        tcq = ops[q]["tc"]()
                        if qa in tas:
                            dep = InstructionNameOrderedSet()
                            dep.add(tas[qa].ins.name)
                            tcq.ins.add_nosync_dependencies_from(dep)

                    tail(0)
                    tas[0] = ops[0]["ta"]()
                    tail(1)
                    ops[0]["dve"]()
                    tas[1] = ops[1]["ta"]()
                    tail(2)
                    ops[1]["dve"]()
                    tc_after(0, 1)
                    tas[2] = ops[2]["ta"]()
                    tail(3)
                    ops[2]["dve"]()
                    tc_after(1, 2)
                    ops[0]["mulh"]()
                    tas[3] = ops[3]["ta"]()
                    ops[3]["dve"]()
                    tc_after(2, 3)
                    ops[1]["mulh"]()
                    tc_after(3, 3)
                    ops[2]["mulh"]()
                    ops[3]["mulh"]()
                    if dbg_step is not None and (c, t) == dbg_step:
                        nc.sync.dma_start(
                            dbg_w.ap(), wqt[:, pp * 4 * WQ:(pp + 1) * 4 * WQ])
                        nc.sync.dma_start(dbg_h.ap(), h16[:])
                        nc.sync.dma_start(dbg_z.ap(), pz[:, 0:64])
                    return
                for k in range(KH):
                    for q in range(4):
                        for G in range(4):
                            mm(k, q, G)
                if NO_CHAIN:
                    pass
                elif CHAIN_SPLIT13:
                    emit_chain13(pz, pp)
                else:
                    ops = [chain_ops(q, pz, pp) for q in range(4)]
                    if CHAIN_NOSYNC:
                        from concourse.bass import InstructionNameOrderedSet

                        tas = [ops[q]["ta"]() for q in range(4)]
                        dep = InstructionNameOrderedSet()
                        dep.add(tas[3].ins.name)
                        for q in range(4):
                            ops[q]["dve"]()
                        for q in range(4):
                            tcq = ops[q]["tc"]()
                            tcq.ins.add_nosync_dependencies_from(dep)
                        for q in range(4):
                            ops[q]["mulh"]()
                    else:
                        for kind, q in CHAIN_SCHED:
                            ops[q][kind]()
                if dbg_step is not None and (c, t) == dbg_step:
                    nc.sync.dma_start(dbg_w.ap(), wqt[:, pp * 4 * WQ:(pp + 1) * 4 * WQ])
                    nc.sync.dma_start(dbg_h.ap(), h16[:])
                    nc.sync.dma_start(dbg_z.ap(), pz[:, 0:64])

            def rep_body(reset):
                if reset:
                    nc.gpsimd.memset(h16[:], 0.0)
                    nc.gpsimd.memset(wqt[:], 0.0)
                g0 = prep_gen(0)
                for _ in g0:
                    pass
                for c in range(nch):
                    gen = prep_gen(c + 1) if c + 1 < nch else None
                    for t in range(Tc):
                        emit_step(c, t, gen)
                    if gen is not None:
                        for _ in gen:  # finish any leftovers
                            pass

            if reps == 1:
                rep_body(False)
            else:
                with tc.For_i(0, reps):
                    rep_body(True)

            nc.sync.dma_start(ho_t.ap(), h16[:, 0:BL * KH])
            # s' is double-written to both parities; read parity 0
            cv = wqt[:, 0:4 * WQ].rearrange("p (q w) -> p q w", w=WQ)[:, :, 0:BL]
            nc.vector.tensor_copy(
                co_sb[:].rearrange("p (q b) -> p q b", b=BL), cv
            )
            nc.sync.dma_start(co_t.ap(), co_sb[:])

    return nc


_CACHE = {}


def _get_compiled(stat="f16", T_steps=T, Tc=128, reps=1):
    key = (stat, T_steps, Tc, reps)
    if key not in _CACHE:
        nc = bacc.Bacc(None, target_bir_lowering=False)
        build_program_v3(nc, T_steps, Tc, reps=reps, stat=stat)
        nc.compile()
        _CACHE[key] = nc
    return _CACHE[key]


def make_tok_idx(tokens_slice, T_steps=T):
    flat = tokens_slice.T.reshape(-1)
    return np.ascontiguousarray(
        flat.reshape(T_steps * BL // P, P).T.astype(np.int32)
    )


def unpack_state(arr):
    a = np.asarray(arr).astype(np.float32).reshape(P, KH, BL)
    return a.transpose(2, 1, 0).reshape(BL, H)


def _make_in_maps(np_inputs):
    tokens = np.ascontiguousarray(np.asarray(np_inputs["tokens"]), dtype=np.int32)
    in_maps = []
    for i in range(NCORES):
        in_maps.append(
            {
                "emb": np.asarray(np_inputs["emb"], np.float32),
                "W": np.asarray(np_inputs["W"], np.float32),
                "U": np.asarray(np_inputs["U"], np.float32),
                "bvec": np.asarray(np_inputs["b"], np.float32),
                "tok": make_tok_idx(tokens[i * BL:(i + 1) * BL]),
            }
        )
    return in_maps


def kernel(tokens, emb, W, U, b, stat="f16"):
    np_inputs = {"tokens": tokens, "emb": emb, "W": W, "U": U, "b": b}
    nc = _get_compiled(stat)
    in_maps = _make_in_maps(np_inputs)
    res = run_bass_kernel_spmd(nc, in_maps, core_ids=list(range(NCORES))).results
    h = np.zeros((B, H), np.float32)
    c = np.zeros((B, H), np.float32)
    for i in range(NCORES):
        h[i * BL:(i + 1) * BL] = unpack_state(res[i]["ho"]) * 0.5
        c[i * BL:(i + 1) * BL] = unpack_state(res[i]["co"]) * 0.5
    return h, c
